# revision 26
# baseline (speedup 1.0000x reference)
"""Supervised-contrastive loss on 8 Trainium2 NeuronCores.

Math (reference):
    z = x / max(||x||, 1e-8)                  row-normalize
    sim = (z @ z.T) / TEMP                    [N, N]
    per-anchor: pos-mean over same-class (excl. self) and logsumexp over
    j != i, then per-class mean, then mean over classes.

exp(sim) is symmetric, so only half the matrix is computed ("wrapped
diagonal band"): anchors are split into 64 chunks of 128 rows; row-chunk
t computes column-chunks d = 0..32 ahead of it (mod 64).  A pair (i, j)
with chunk distance d is computed once (at the nearer row) for
1 <= d <= 31, at both rows for d == 32 -- the d=32 cell's exp carries
bias = -ln2 so each side contributes exactly half.  Row sums over the
band ride on the ScalarE Exp via accum_out; the "missing" transposed
halves are recovered as column sums: each exp tile (bf16, SBUF) is
added by the DVE into a per-core [128, 8192] accumulator, which is
DMA'd out raw and partition-reduced on the host.

Core c owns row-chunks t = c + 8k (k = 0..7).  Its z8 copy is
column-rotated by 128*c on the host so the band's SBUF addresses are
identical on every core (SPMD shares one instruction stream).  Class-
segment sums come from a small GEMM tm = A @ W.T with W[c] = sum of
z8 rows of class c (host-precomputed), so no masking is needed.  The
diagonal sim[i,i] = ||z8[i]||^2 is reconstructed exactly on host and
subtracted there.

Layout: all fp8 operands are host-packed for DoubleRow so that feature
d = kk*256 + i*128 + p lands on partition p, plane i of contraction tile
kk, giving 2KB-contiguous per-partition DMA descriptors.

Hardware notes baked into this structure: DMAs only from nc.sync,
one matmul accumulation group per PSUM bank, fp8 DoubleRow streams
1 output element per cycle per 256-deep pass (157 TF/s peak), ScalarE
is 1 elem/lane/cycle at 1.2 GHz (the old full-matrix kernel was
bottlenecked by it), and the d=32 runt cells are deferred to a tail
phase so the two rotating [128, 2048] PSUM slots never stall the PE
inside the main 8-row loop.
"""

import math

import numpy as np
import ml_dtypes

N = 8192           # anchors
D = 768            # feature dim
NOP = 64           # number of classes
CORES = 8
KT8 = D // 256     # 3 double-row contraction tiles
NROW = 8           # 128-row chunks per core
CELLW = 2048       # wide cell width (one PSUM slot, 4 banks)
RUNTW = 128        # d=32 runt cell width
BANDW = 33 * 128   # 4224 cols per row-chunk (d = 0..32)
GW = 2048          # z8 DMA group width
NG = N // GW       # 4 groups
TEMP_INV = 10.0
EPS = 1e-8

FP8 = ml_dtypes.float8_e4m3
BF16 = ml_dtypes.bfloat16

_CACHE = {}
LAST_RESULT = None  # BassKernelResults of the most recent run (for profiling)


def _splits(start, width):
    """Split a rotated-coords col range into <=2 non-wrapping pieces."""
    start %= N
    if start + width <= N:
        return [(start, width)]
    return [(start, N - start), (0, start + width - N)]


# Runt processing order k = 4..7, 0..3 touches acc slices 0, 2, 4, ...
# 14 in ascending order; odd slices are final after the main rows.  So
# after the i-th runt the [1024*i, 1024*(i+1)) acc region is final.
RUNT_ORDER = [4, 5, 6, 7, 0, 1, 2, 3]

# Main cells ordered by band end-column: fresh z8 demand is a uniform
# 1024 cols/cell, matching the DMA supply rate (no mid-loop stalls).
STAIR = [(0, 0), (1, 0), (0, 1), (2, 0), (1, 1), (3, 0), (2, 1), (4, 0),
         (3, 1), (5, 0), (4, 1), (6, 0), (5, 1), (7, 0), (6, 1), (7, 1)]


def _build_nc():
    from concourse import bacc
    import concourse.mybir as mybir
    import concourse.tile as tile

    f8 = mybir.dt.float8e4
    f32 = mybir.dt.float32
    bf16 = mybir.dt.bfloat16
    Exp = mybir.ActivationFunctionType.Exp
    DR = mybir.MatmulPerfMode.DoubleRow

    nc = bacc.Bacc(
        "TRN2", target_bir_lowering=False, debug=False, enable_asserts=False
    )
    z8 = nc.dram_tensor("z8", [128, NG, KT8, 2, GW], f8, kind="ExternalInput").ap()
    a8 = nc.dram_tensor("a8", [128, NROW, KT8, 2, 128], f8, kind="ExternalInput").ap()
    w8 = nc.dram_tensor("w8", [128, KT8, 2, NOP], f8, kind="ExternalInput").ap()
    tm = nc.dram_tensor("tm", [128, NROW, NOP], f32, kind="ExternalOutput").ap()
    pacc = nc.dram_tensor("pacc", [128, NROW, 3], f32, kind="ExternalOutput").ap()
    acc_out = nc.dram_tensor("acc_out", [128, N], bf16, kind="ExternalOutput").ap()

    with tile.TileContext(nc) as tc:
        with (
            tc.tile_pool(name="zin", bufs=NG) as zin,
            tc.tile_pool(name="epool", bufs=3) as epool,
            tc.tile_pool(name="singles", bufs=1) as singles,
        ):
            # ---- input DMAs (all contiguous per-partition), ordered for
            # earliest row-0 start: a8 row 0, z8 g0, g1, a8 rows 1-7,
            # g2, g3, w8 ----
            a8_sb = singles.tile([128, NROW, KT8, 2, 128], f8)
            nc.sync.dma_start(out=a8_sb[:, :1], in_=a8[:, :1])
            z8_sb = {}

            def dma_z8(g):
                z8_t = zin.tile([128, KT8, 2, GW], f8, name="z8_t", tag="z8_t")
                nc.sync.dma_start(
                    out=z8_t.rearrange("p a b c -> p (a b c)"),
                    in_=z8[:, g].rearrange("p a b c -> p (a b c)"),
                )
                z8_sb[g] = z8_t

            dma_z8(0)
            dma_z8(1)
            nc.sync.dma_start(out=a8_sb[:, 1:], in_=a8[:, 1:])
            dma_z8(2)
            dma_z8(3)
            w8_sb = singles.tile([128, KT8, 2, NOP], f8)
            nc.sync.dma_start(out=w8_sb, in_=w8)

            # colsum accumulator, zeroed while DMAs stream in
            acc = singles.tile([128, N], bf16)
            nc.vector.memset(acc, 0.0)

            # bias = -ln2 for the d=32 runt cells (halves their exp)
            nln2 = singles.tile([128, 1], f32)
            nc.vector.memset(nln2, -math.log(2.0))

            pacc_sb = singles.tile([128, NROW, 3], f32)
            tm_sb = singles.tile([128, NROW, NOP], f32)

            ps_pool = tc.alloc_tile_pool(name="ps", bufs=2, space="PSUM")

            def do_cell(k, ci, start, w, bias):
                """One band cell: sim matmuls -> Exp(+rowsum) -> DVE colsum."""
                ps_t = ps_pool.tile([128, w], f32, name="ps_t", tag="ps_t")
                for kk in range(KT8):
                    lhsT = a8_sb[:, k, kk]
                    for jj in range(0, w, 512):
                        sw = min(512, w - jj)
                        g, off = divmod((start + jj) % N, GW)
                        nc.tensor.matmul(
                            ps_t[:, jj:jj + sw],
                            lhsT,
                            z8_sb[g][:, kk, :, off:off + sw],
                            start=(kk == 0),
                            stop=(kk == KT8 - 1),
                            perf_mode=DR,
                        )
                e_t = epool.tile([128, w], bf16, name="e_t", tag="e_t")
                nc.scalar.activation(
                    out=e_t,
                    in_=ps_t,
                    func=Exp,
                    scale=TEMP_INV,
                    bias=bias,
                    accum_out=pacc_sb[:, k, ci:ci + 1],
                )
                # colsum: skip the d=0 (diagonal) chunk at the band head
                eoff = RUNTW if ci == 0 else 0
                for s0, sw in _splits(start + eoff, w - eoff):
                    e0 = (s0 - start) % N
                    nc.vector.tensor_add(
                        acc[:, s0:s0 + sw],
                        acc[:, s0:s0 + sw],
                        e_t[:, e0:e0 + sw],
                    )

            # ---- main band: two wide cells per row-chunk, in staircase
            # (end-column) order so z8 demand tracks DMA supply ----
            for k, ci in STAIR:
                do_cell(k, ci, 1024 * k + ci * CELLW, CELLW, 0.0)
            nc.sync.dma_start(out=pacc[:, :, :2], in_=pacc_sb[:, :, :2])

            # ---- d=32 runt cells (halved via bias=-ln2), ordered so the
            # acc out-DMA regions become final left to right and ship
            # behind the runt chain; the tm GEMM (tm[:, k, c] = A_k @ W.T)
            # interleaves with the runts to hide its serial mm->copy chain
            for i, k in enumerate(RUNT_ORDER):
                do_cell(k, 2, 1024 * k + 4096, RUNTW, nln2)
                nc.sync.dma_start(
                    out=acc_out[:, 1024 * i:1024 * (i + 1)],
                    in_=acc[:, 1024 * i:1024 * (i + 1)],
                )
                pst = ps_pool.tile([128, NOP], f32, name="ps_t", tag="ps_t")
                for kk in range(KT8):
                    nc.tensor.matmul(
                        pst,
                        a8_sb[:, k, kk],
                        w8_sb[:, kk, :, :],
                        start=(kk == 0),
                        stop=(kk == KT8 - 1),
                        perf_mode=DR,
                    )
                nc.vector.tensor_copy(tm_sb[:, k, :], pst)
            nc.sync.dma_start(out=tm, in_=tm_sb)
            ps_pool.release()

            nc.sync.dma_start(out=pacc[:, :, 2:], in_=pacc_sb[:, :, 2:])

    nc.compile()
    return nc


def _get_nc():
    if "nc" not in _CACHE:
        _CACHE["nc"] = _build_nc()
    return _CACHE["nc"]


def _pack_dr(mat_t):
    """[D, cols] -> [128, KT8, 2, cols] with d = kk*256 + i*128 + p."""
    d, cols = mat_t.shape
    return np.ascontiguousarray(
        mat_t.reshape(KT8, 2, 128, cols).transpose(2, 0, 1, 3)
    )


def kernel(x, op_ids, n_op):
    global LAST_RESULT
    from concourse.bass_utils import run_bass_kernel_spmd

    x = np.asarray(x, dtype=np.float32).reshape(-1, D)
    op_ids = np.asarray(op_ids).reshape(-1).astype(np.int64)
    n_op_i = int(np.asarray(n_op))

    # ---- host prep: normalize, quantize, class sums, diagonal ----
    norms = np.sqrt((x.astype(np.float64) ** 2).sum(axis=1))
    norms = np.maximum(norms, EPS).astype(np.float32)
    z = x / norms[:, None]

    z8 = z.astype(FP8)
    z8f = z8.astype(np.float32)

    onehot = np.zeros((N, NOP), np.float32)
    onehot[np.arange(N), op_ids] = 1.0
    W8 = (onehot.T @ z8f).astype(FP8)               # [NOP, D] fp8

    z8_packed = _pack_dr(np.ascontiguousarray(z8.T))          # [128,3,2,N]
    w8_packed = _pack_dr(np.ascontiguousarray(W8.T.astype(FP8)))
    ssq = (z8f.astype(np.float64) ** 2).sum(axis=1)  # = sim[i, i]

    in_maps = []
    for c in range(CORES):
        rows = np.concatenate(
            [np.arange(128 * (c + 8 * k), 128 * (c + 8 * k) + 128)
             for k in range(NROW)]
        )
        # [128, NROW, KT8, 2, 128]: per-row-chunk contiguous lhsT blocks
        a8_c = np.ascontiguousarray(
            z8_packed[:, :, :, rows]
            .reshape(128, KT8, 2, NROW, 128)
            .transpose(0, 3, 1, 2, 4)
        )
        zrot = np.roll(z8_packed, -128 * c, axis=3)
        z8_c = np.ascontiguousarray(
            zrot.reshape(128, KT8, 2, NG, GW).transpose(0, 3, 1, 2, 4)
        )
        in_maps.append({"z8": z8_c, "a8": a8_c, "w8": w8_packed})

    nc = _get_nc()
    res = run_bass_kernel_spmd(nc, in_maps, core_ids=list(range(CORES)))
    LAST_RESULT = res

    # ---- host post: assemble es = rowsums + colsums, finish loss ----
    es = np.zeros(N, np.float64)
    tm_full = np.zeros((N, NOP), np.float64)
    for c in range(CORES):
        r = res.results[c]
        pacc_c = r["pacc"].astype(np.float64)      # [128, NROW, 3]
        tm_c = r["tm"].astype(np.float64)          # [128, NROW, NOP]
        cs = r["acc_out"].astype(np.float64).sum(axis=0)  # [8192] rotated
        es += np.roll(cs, 128 * c)                 # unrotate
        for k in range(NROW):
            t = c + 8 * k
            rows = slice(128 * t, 128 * t + 128)
            es[rows] += pacc_c[:, k, :].sum(axis=1)
            tm_full[rows] = tm_c[:, k, :]

    lse = np.log(es - np.exp(TEMP_INV * ssq))
    pos_sum = TEMP_INV * (tm_full[np.arange(N), op_ids] - ssq)
    counts = np.bincount(op_ids, minlength=n_op_i).astype(np.float64)
    pos_cnt = counts[op_ids] - 1.0

    loss_i = np.where(pos_cnt > 0, -pos_sum / np.maximum(pos_cnt, 1.0) + lse, 0.0)
    cls_sum = np.bincount(op_ids, weights=loss_i, minlength=n_op_i)
    cls_loss = np.where(counts > 0, cls_sum / np.maximum(counts, 1.0), 0.0)
    return np.float32(cls_loss.mean())


# revision 33
# speedup vs baseline: 1.1870x; 1.1870x over previous
"""Supervised-contrastive loss on 8 Trainium2 NeuronCores.

Math (reference):
    z = x / max(||x||, 1e-8)                  row-normalize
    sim = (z @ z.T) / TEMP                    [N, N]
    per-anchor: pos-mean over same-class (excl. self) and logsumexp over
    j != i, then per-class mean, then mean over classes.

exp(sim) is symmetric, so only half the matrix is computed ("wrapped
diagonal band"): anchors are split into 64 chunks of 128 rows; row-chunk
t computes column-chunks d = 0..32 ahead of it (mod 64).  A pair (i, j)
with chunk distance d is computed once (at the nearer row) for
1 <= d <= 31, at both rows for d == 32 -- the d=32 cell's exp carries
bias = -ln2 so each side contributes exactly half.  Row sums over the
band ride on the ScalarE Exp via accum_out; the "missing" transposed
halves are recovered as column sums: each exp tile (bf16, SBUF) is
added by the DVE into a per-core [128, 5120] accumulator, which is
DMA'd out raw and partition-reduced on the host.

Core c owns the CONSECUTIVE row-chunks t = 8c + k (k = 0..7); its z8
copy is column-rotated by 1024*c on the host, so the union of its
bands is rotated cols [0, 5120) -- only 6144 rotated columns of z8 are
shipped (the band never wraps) and the SBUF addresses are identical on
every core (SPMD shares one instruction stream).  Class-segment sums
come from a small GEMM tm = A @ W.T with W[c] = sum of z8 rows of
class c (host-precomputed).  The diagonal sim[i,i] = ||z8[i]||^2 is
reconstructed exactly on host and subtracted there.

Layout: all fp8 operands are host-packed for DoubleRow so that feature
d = kk*256 + i*128 + p lands on partition p, plane i of contraction
tile kk; every DMA is per-partition contiguous (strided DGE issues
cost ~3x on the sync sequencer).

Hardware notes baked into this structure: DMAs only from nc.sync, one
matmul accumulation group per PSUM bank, fp8 DoubleRow streams 1
output element per cycle per 256-deep pass (157 TF/s peak), ScalarE is
1 elem/lane/cycle at 1.2 GHz (the old full-matrix kernel was
bottlenecked on it), DMA moves only ~0.3 MB/us end-to-end (input
volume is minimized and output regions ship as soon as their last
writer retires), and the d=32 runt cells run as a tail phase so the
two rotating [128, 2048] PSUM slots never stall the PE mid-loop.
"""

import math

import numpy as np
import ml_dtypes

N = 8192           # anchors
D = 768            # feature dim
NOP = 64           # number of classes
CORES = 8
KT8 = D // 256     # 3 double-row contraction tiles
NROW = 8           # 128-row chunks per core
CELLW = 2048       # wide cell width (one PSUM slot, 4 banks)
RUNTW = 128        # d=32 runt cell width
ZCOLS = 5120       # rotated z8 columns shipped per core (5 groups)
NGZ = 5
ACCW = 5120        # rotated colsum extent per core
GW = 1024          # z8 group width
TEMP_INV = 10.0
EPS = 1e-8

FP8 = ml_dtypes.float8_e4m3

_CACHE = {}
LAST_RESULT = None  # BassKernelResults of the most recent run (for profiling)


def _build_nc():
    from concourse import bacc
    import concourse.mybir as mybir
    import concourse.tile as tile

    f8 = mybir.dt.float8e4
    f32 = mybir.dt.float32
    bf16 = mybir.dt.bfloat16
    Exp = mybir.ActivationFunctionType.Exp
    DR = mybir.MatmulPerfMode.DoubleRow

    nc = bacc.Bacc(
        "TRN2", target_bir_lowering=False, debug=False, enable_asserts=False
    )
    z8 = nc.dram_tensor("z8", [128, NGZ, KT8, 2, GW], f8, kind="ExternalInput").ap()
    a8 = nc.dram_tensor("a8", [128, NROW, KT8, 2, 128], f8, kind="ExternalInput").ap()
    w8 = nc.dram_tensor("w8", [128, KT8, 2, NOP], f8, kind="ExternalInput").ap()
    tm = nc.dram_tensor("tm", [128, NROW, NOP], f32, kind="ExternalOutput").ap()
    pacc = nc.dram_tensor("pacc", [128, NROW, 4], f32, kind="ExternalOutput").ap()
    acc_out = nc.dram_tensor("acc_out", [128, ACCW], bf16, kind="ExternalOutput").ap()

    with tile.TileContext(nc) as tc:
        with (
            tc.tile_pool(name="zin", bufs=NGZ) as zin,
            tc.tile_pool(name="epool", bufs=3) as epool,
            tc.tile_pool(name="singles", bufs=1) as singles,
        ):
            # ---- input DMAs (all per-partition contiguous), ordered for
            # earliest row-0 start ----
            a8_sb = singles.tile([128, NROW, KT8, 2, 128], f8)
            nc.sync.dma_start(out=a8_sb[:, :3], in_=a8[:, :3])
            z8_sb = {}

            def dma_z8(g):
                z8_t = zin.tile([128, KT8, 2, GW], f8, name="z8_t", tag="z8_t")
                nc.sync.dma_start(
                    out=z8_t.rearrange("p a b c -> p (a b c)"),
                    in_=z8[:, g].rearrange("p a b c -> p (a b c)"),
                )
                z8_sb[g] = z8_t

            dma_z8(0)
            dma_z8(1)
            dma_z8(2)
            nc.sync.dma_start(out=a8_sb[:, 3:], in_=a8[:, 3:])
            dma_z8(3)
            dma_z8(4)
            w8_sb = singles.tile([128, KT8, 2, NOP], f8)
            nc.sync.dma_start(out=w8_sb, in_=w8)

            # colsum accumulator + rowsum slots, zeroed during the DMA fill
            acc = singles.tile([128, ACCW], bf16)
            nc.vector.memset(acc, 0.0)
            pacc_sb = singles.tile([128, NROW, 4], f32)
            nc.vector.memset(pacc_sb.rearrange("p a b -> p (a b)"), 0.0)
            tm_sb = singles.tile([128, NROW, NOP], f32)

            # bias = -ln2 for the d=32 runt cells (halves their exp)
            nln2 = singles.tile([128, 1], f32)
            nc.vector.memset(nln2, -math.log(2.0))

            ps_pool = tc.alloc_tile_pool(name="ps", bufs=2, space="PSUM")

            def do_cell(k, slot, start, w, bias, skip_head):
                """One band cell: sim matmuls -> Exp(+rowsum) -> DVE colsum.

                skip_head: first 128 cols are the d=0 diagonal chunk,
                excluded from the colsum accumulator.
                """
                ps_t = ps_pool.tile([128, w], f32, name="ps_t", tag="ps_t")
                # 512-col slices; a slice crossing a z8-group boundary is
                # split into pieces.  matmul start resets the whole PSUM
                # bank, so only the bank's FIRST matmul carries start=True
                # (the reset zeroes the later pieces' region too).
                for jj in range(0, w, 512):
                    pieces = []
                    o = 0
                    while o < min(512, w - jj):
                        g, off = divmod(start + jj + o, GW)
                        sw = min(512 - o, w - jj - o, GW - off)
                        pieces.append((o, g, off, sw))
                        o += sw
                    for kk in range(KT8):
                        lhsT = a8_sb[:, k, kk]
                        for pi, (o, g, off, sw) in enumerate(pieces):
                            nc.tensor.matmul(
                                ps_t[:, jj + o:jj + o + sw],
                                lhsT,
                                z8_sb[g][:, kk, :, off:off + sw],
                                start=(kk == 0 and pi == 0),
                                stop=(kk == KT8 - 1 and pi == len(pieces) - 1),
                                perf_mode=DR,
                                skip_group_check=True,
                            )
                e_t = epool.tile([128, w], bf16, name="e_t", tag="e_t")
                nc.scalar.activation(
                    out=e_t,
                    in_=ps_t,
                    func=Exp,
                    scale=TEMP_INV,
                    bias=bias,
                    accum_out=pacc_sb[:, k, slot:slot + 1],
                )
                eoff = RUNTW if skip_head else 0
                if w > eoff:
                    s0 = start + eoff
                    nc.vector.tensor_add(
                        acc[:, s0:s0 + w - eoff],
                        acc[:, s0:s0 + w - eoff],
                        e_t[:, eoff:w],
                    )

            # ---- main band: near cells (d 0..15) for all rows, then far
            # cells (d 16..31); row 0's first cell is split so compute
            # starts before all of z8 group 0 has landed ----
            do_cell(0, 0, 0, 1024, 0.0, True)
            do_cell(0, 1, 1024, 1024, 0.0, False)
            for k in range(1, NROW):
                do_cell(k, 0, 128 * k, CELLW, 0.0, True)
                if k == 6:
                    nc.sync.dma_start(out=acc_out[:, :1024], in_=acc[:, :1024])
                if k == 7:
                    nc.sync.dma_start(
                        out=acc_out[:, 1024:2048], in_=acc[:, 1024:2048]
                    )
            for k in range(NROW):
                do_cell(k, 2, 128 * k + CELLW, CELLW, 0.0, False)
            nc.sync.dma_start(out=acc_out[:, 2048:4096], in_=acc[:, 2048:4096])

            # ---- d=32 runt cells (halved via bias=-ln2) ----
            for k in range(NROW):
                do_cell(k, 3, 128 * k + 4096, RUNTW, nln2, False)
            nc.sync.dma_start(out=acc_out[:, 4096:ACCW], in_=acc[:, 4096:ACCW])
            nc.sync.dma_start(out=pacc, in_=pacc_sb)
            ps_pool.release()

            # ---- class-segment sums tm[:, k, c] = A_k @ W.T in a 4-deep
            # pool (overlaps the acc DMA drain) ----
            tm_pool = tc.alloc_tile_pool(name="tmp", bufs=4, space="PSUM")
            for k in range(NROW):
                pst = tm_pool.tile([128, NOP], f32, name="tm_t", tag="tm_t")
                for kk in range(KT8):
                    nc.tensor.matmul(
                        pst,
                        a8_sb[:, k, kk],
                        w8_sb[:, kk, :, :],
                        start=(kk == 0),
                        stop=(kk == KT8 - 1),
                        perf_mode=DR,
                    )
                nc.vector.tensor_copy(tm_sb[:, k, :], pst)
            nc.sync.dma_start(out=tm, in_=tm_sb)
            tm_pool.release()

    nc.compile()
    return nc


def _get_nc():
    if "nc" not in _CACHE:
        _CACHE["nc"] = _build_nc()
    return _CACHE["nc"]


def _pack_dr(mat_t):
    """[D, cols] -> [128, KT8, 2, cols] with d = kk*256 + i*128 + p."""
    d, cols = mat_t.shape
    return np.ascontiguousarray(
        mat_t.reshape(KT8, 2, 128, cols).transpose(2, 0, 1, 3)
    )


def kernel(x, op_ids, n_op):
    global LAST_RESULT
    from concourse.bass_utils import run_bass_kernel_spmd

    x = np.asarray(x, dtype=np.float32).reshape(-1, D)
    op_ids = np.asarray(op_ids).reshape(-1).astype(np.int64)
    n_op_i = int(np.asarray(n_op))

    # ---- host prep: normalize, quantize, class sums, diagonal ----
    norms = np.sqrt((x.astype(np.float64) ** 2).sum(axis=1))
    norms = np.maximum(norms, EPS).astype(np.float32)
    z = x / norms[:, None]

    z8 = z.astype(FP8)
    z8f = z8.astype(np.float32)

    onehot = np.zeros((N, NOP), np.float32)
    onehot[np.arange(N), op_ids] = 1.0
    W8 = (onehot.T @ z8f).astype(FP8)               # [NOP, D] fp8

    z8_packed = _pack_dr(np.ascontiguousarray(z8.T))          # [128,3,2,N]
    w8_packed = _pack_dr(np.ascontiguousarray(W8.T.astype(FP8)))
    ssq = (z8f.astype(np.float64) ** 2).sum(axis=1)  # = sim[i, i]

    in_maps = []
    for c in range(CORES):
        # rows 1024c..1024c+1023 as [128, NROW, KT8, 2, 128] lhsT blocks
        a8_c = np.ascontiguousarray(
            z8_packed[:, :, :, 1024 * c:1024 * (c + 1)]
            .reshape(128, KT8, 2, NROW, 128)
            .transpose(0, 3, 1, 2, 4)
        )
        # rotated z8 columns [0, ZCOLS) as NGZ contiguous groups
        idx = (np.arange(ZCOLS) + 1024 * c) % N
        z8_c = np.ascontiguousarray(
            z8_packed[:, :, :, idx]
            .reshape(128, KT8, 2, NGZ, GW)
            .transpose(0, 3, 1, 2, 4)
        )
        in_maps.append({"z8": z8_c, "a8": a8_c, "w8": w8_packed})

    nc = _get_nc()
    res = run_bass_kernel_spmd(nc, in_maps, core_ids=list(range(CORES)))
    LAST_RESULT = res

    # ---- host post: assemble es = rowsums + colsums, finish loss ----
    es = np.zeros(N, np.float64)
    tm_full = np.zeros((N, NOP), np.float64)
    for c in range(CORES):
        r = res.results[c]
        pacc_c = r["pacc"].astype(np.float64)      # [128, NROW, 4]
        tm_c = r["tm"].astype(np.float64)          # [128, NROW, NOP]
        cs = np.zeros(N, np.float64)
        cs[:ACCW] = r["acc_out"].astype(np.float64).sum(axis=0)
        es += np.roll(cs, 1024 * c)                # unrotate
        for k in range(NROW):
            t = 8 * c + k
            rows = slice(128 * t, 128 * t + 128)
            es[rows] += pacc_c[:, k, :].sum(axis=1)
            tm_full[rows] = tm_c[:, k, :]

    lse = np.log(es - np.exp(TEMP_INV * ssq))
    pos_sum = TEMP_INV * (tm_full[np.arange(N), op_ids] - ssq)
    counts = np.bincount(op_ids, minlength=n_op_i).astype(np.float64)
    pos_cnt = counts[op_ids] - 1.0

    loss_i = np.where(pos_cnt > 0, -pos_sum / np.maximum(pos_cnt, 1.0) + lse, 0.0)
    cls_sum = np.bincount(op_ids, weights=loss_i, minlength=n_op_i)
    cls_loss = np.where(counts > 0, cls_sum / np.maximum(counts, 1.0), 0.0)
    return np.float32(cls_loss.mean())


# revision 38
# speedup vs baseline: 1.1886x; 1.0014x over previous
"""Supervised-contrastive loss on 8 Trainium2 NeuronCores.

Math (reference):
    z = x / max(||x||, 1e-8)                  row-normalize
    sim = (z @ z.T) / TEMP                    [N, N]
    per-anchor: pos-mean over same-class (excl. self) and logsumexp over
    j != i, then per-class mean, then mean over classes.

exp(sim) is symmetric, so only half the matrix is computed ("wrapped
diagonal band"): anchors are split into 64 chunks of 128 rows; row-chunk
t computes column-chunks d = 0..32 ahead of it (mod 64).  A pair (i, j)
with chunk distance d is computed once (at the nearer row) for
1 <= d <= 31, at both rows for d == 32 -- the d=32 cell's exp carries
bias = -ln2 so each side contributes exactly half.  Row sums over the
band ride on the ScalarE Exp via accum_out; the "missing" transposed
halves are recovered as column sums: each exp tile (bf16, SBUF) is
added by the DVE into a per-core [128, 5120] accumulator, which is
DMA'd out raw and partition-reduced on the host.

Core c owns the CONSECUTIVE row-chunks t = 8c + k (k = 0..7); its z8
copy is column-rotated by 1024*c on the host, so the union of its
bands is rotated cols [0, 5120) -- only 6144 rotated columns of z8 are
shipped (the band never wraps) and the SBUF addresses are identical on
every core (SPMD shares one instruction stream).  Class-segment sums
come from a small GEMM tm = A @ W.T with W[c] = sum of z8 rows of
class c (host-precomputed).  The diagonal sim[i,i] = ||z8[i]||^2 is
reconstructed exactly on host and subtracted there.

Layout: all fp8 operands are host-packed for DoubleRow so that feature
d = kk*256 + i*128 + p lands on partition p, plane i of contraction
tile kk; every DMA is per-partition contiguous (strided DGE issues
cost ~3x on the sync sequencer).

Hardware notes baked into this structure: DMAs only from nc.sync, one
matmul accumulation group per PSUM bank, fp8 DoubleRow streams 1
output element per cycle per 256-deep pass (157 TF/s peak), ScalarE is
1 elem/lane/cycle at 1.2 GHz (the old full-matrix kernel was
bottlenecked on it), DMA moves only ~0.3 MB/us end-to-end (input
volume is minimized and output regions ship as soon as their last
writer retires), and the d=32 runt cells run as a tail phase so the
two rotating [128, 2048] PSUM slots never stall the PE mid-loop.
"""

import math

import numpy as np
import ml_dtypes

N = 8192           # anchors
D = 768            # feature dim
NOP = 64           # number of classes
CORES = 8
KT8 = D // 256     # 3 double-row contraction tiles
NROW = 8           # 128-row chunks per core
CELLW = 2048       # wide cell width (one PSUM slot, 4 banks)
RUNTW = 128        # d=32 runt cell width
ZCOLS = 5120       # rotated z8 columns shipped per core (5 groups)
NGZ = 5
ACCW = 5120        # rotated colsum extent per core
GW = 1024          # z8 group width
TEMP_INV = 10.0
EPS = 1e-8

FP8 = ml_dtypes.float8_e4m3

_CACHE = {}
LAST_RESULT = None  # BassKernelResults of the most recent run (for profiling)


def _build_nc():
    from concourse import bacc
    import concourse.mybir as mybir
    import concourse.tile as tile

    f8 = mybir.dt.float8e4
    f32 = mybir.dt.float32
    bf16 = mybir.dt.bfloat16
    Exp = mybir.ActivationFunctionType.Exp
    DR = mybir.MatmulPerfMode.DoubleRow

    nc = bacc.Bacc(
        "TRN2", target_bir_lowering=False, debug=False, enable_asserts=False
    )
    z8 = nc.dram_tensor("z8", [128, NGZ, KT8, 2, GW], f8, kind="ExternalInput").ap()
    a8 = nc.dram_tensor("a8", [128, NROW, KT8, 2, 128], f8, kind="ExternalInput").ap()
    w8 = nc.dram_tensor("w8", [128, KT8, 2, NOP], f8, kind="ExternalInput").ap()
    tm = nc.dram_tensor("tm", [128, NROW, NOP], f32, kind="ExternalOutput").ap()
    pacc = nc.dram_tensor("pacc", [128, NROW, 4], f32, kind="ExternalOutput").ap()
    acc_out = nc.dram_tensor("acc_out", [128, ACCW], bf16, kind="ExternalOutput").ap()

    with tile.TileContext(nc) as tc:
        with (
            tc.tile_pool(name="zin", bufs=NGZ) as zin,
            tc.tile_pool(name="epool", bufs=3) as epool,
            tc.tile_pool(name="singles", bufs=1) as singles,
        ):
            # ---- input DMAs (all per-partition contiguous), ordered for
            # earliest row-0 start ----
            a8_sb = singles.tile([128, NROW, KT8, 2, 128], f8)
            nc.sync.dma_start(out=a8_sb[:, :3], in_=a8[:, :3])
            w8_sb = singles.tile([128, KT8, 2, NOP], f8)
            nc.sync.dma_start(out=w8_sb, in_=w8)
            z8_sb = {}

            def dma_z8(g):
                z8_t = zin.tile([128, KT8, 2, GW], f8, name="z8_t", tag="z8_t")
                nc.sync.dma_start(
                    out=z8_t.rearrange("p a b c -> p (a b c)"),
                    in_=z8[:, g].rearrange("p a b c -> p (a b c)"),
                )
                z8_sb[g] = z8_t

            dma_z8(0)
            dma_z8(1)
            dma_z8(2)
            nc.sync.dma_start(out=a8_sb[:, 3:], in_=a8[:, 3:])
            dma_z8(3)
            dma_z8(4)

            # colsum accumulator + rowsum slots, zeroed during the DMA fill
            acc = singles.tile([128, ACCW], bf16)
            nc.vector.memset(acc, 0.0)
            pacc_sb = singles.tile([128, NROW, 4], f32)
            nc.vector.memset(pacc_sb.rearrange("p a b -> p (a b)"), 0.0)
            tm_sb = singles.tile([128, NROW, NOP], f32)

            # bias = -ln2 for the d=32 runt cells (halves their exp)
            nln2 = singles.tile([128, 1], f32)
            nc.vector.memset(nln2, -math.log(2.0))

            ps_pool = tc.alloc_tile_pool(name="ps", bufs=2, space="PSUM")

            def do_cell(k, slot, start, w, bias, skip_head):
                """One band cell: sim matmuls -> Exp(+rowsum) -> DVE colsum.

                skip_head: first 128 cols are the d=0 diagonal chunk,
                excluded from the colsum accumulator.
                """
                ps_t = ps_pool.tile([128, w], f32, name="ps_t", tag="ps_t")
                # 512-col slices; a slice crossing a z8-group boundary is
                # split into pieces.  matmul start resets the whole PSUM
                # bank, so only the bank's FIRST matmul carries start=True
                # (the reset zeroes the later pieces' region too).  kk
                # outer so the lhsT weights are reused across the slices.
                slices = []
                for jj in range(0, w, 512):
                    pieces = []
                    o = 0
                    while o < min(512, w - jj):
                        g, off = divmod(start + jj + o, GW)
                        sw = min(512 - o, w - jj - o, GW - off)
                        pieces.append((jj + o, g, off, sw))
                        o += sw
                    slices.append(pieces)
                for kk in range(KT8):
                    lhsT = a8_sb[:, k, kk]
                    for pieces in slices:
                        for pi, (o, g, off, sw) in enumerate(pieces):
                            nc.tensor.matmul(
                                ps_t[:, o:o + sw],
                                lhsT,
                                z8_sb[g][:, kk, :, off:off + sw],
                                start=(kk == 0 and pi == 0),
                                stop=(kk == KT8 - 1 and pi == len(pieces) - 1),
                                perf_mode=DR,
                                skip_group_check=True,
                            )
                e_t = epool.tile([128, w], bf16, name="e_t", tag="e_t")
                nc.scalar.activation(
                    out=e_t,
                    in_=ps_t,
                    func=Exp,
                    scale=TEMP_INV,
                    bias=bias,
                    accum_out=pacc_sb[:, k, slot:slot + 1],
                )
                eoff = RUNTW if skip_head else 0
                if w > eoff:
                    s0 = start + eoff
                    nc.vector.tensor_add(
                        acc[:, s0:s0 + w - eoff],
                        acc[:, s0:s0 + w - eoff],
                        e_t[:, eoff:w],
                    )

            def do_tm(k, pool, tag):
                pst = pool.tile([128, NOP], f32, name="tm_t", tag=tag)
                for kk in range(KT8):
                    nc.tensor.matmul(
                        pst,
                        a8_sb[:, k, kk],
                        w8_sb[:, kk, :, :],
                        start=(kk == 0),
                        stop=(kk == KT8 - 1),
                        perf_mode=DR,
                    )
                nc.vector.tensor_copy(tm_sb[:, k, :], pst)

            # tm rows 0-2 need only a8 head + w8: free PE work while z8
            # group 0 is still streaming in
            for k in range(3):
                do_tm(k, ps_pool, "ps_t")

            # ---- main band: near cells (d 0..15) for all rows, then far
            # cells (d 16..31); row 0's first cell is split so compute
            # starts before all of z8 group 0 has landed ----
            do_cell(0, 0, 0, 1024, 0.0, True)
            do_cell(0, 1, 1024, 1024, 0.0, False)
            for k in range(1, NROW):
                do_cell(k, 0, 128 * k, CELLW, 0.0, True)
                if k == 6:
                    nc.sync.dma_start(out=acc_out[:, :1024], in_=acc[:, :1024])
                if k == 7:
                    nc.sync.dma_start(
                        out=acc_out[:, 1024:2048], in_=acc[:, 1024:2048]
                    )
            for k in range(NROW):
                do_cell(k, 2, 128 * k + CELLW, CELLW, 0.0, False)
            nc.sync.dma_start(out=acc_out[:, 2048:4096], in_=acc[:, 2048:4096])

            # ---- d=32 runt cells (halved via bias=-ln2) ----
            for k in range(NROW):
                do_cell(k, 3, 128 * k + 4096, RUNTW, nln2, False)
            nc.sync.dma_start(out=acc_out[:, 4096:ACCW], in_=acc[:, 4096:ACCW])
            nc.sync.dma_start(out=pacc, in_=pacc_sb)
            ps_pool.release()

            # ---- tm rows 3-7 in a 5-deep pool (overlaps the acc DMA
            # drain) ----
            tm_pool = tc.alloc_tile_pool(name="tmp", bufs=5, space="PSUM")
            for k in range(3, NROW):
                do_tm(k, tm_pool, "tm_t")
            nc.sync.dma_start(out=tm, in_=tm_sb)
            tm_pool.release()

    nc.compile()
    return nc


def _get_nc():
    if "nc" not in _CACHE:
        _CACHE["nc"] = _build_nc()
    return _CACHE["nc"]


def _pack_dr(mat_t):
    """[D, cols] -> [128, KT8, 2, cols] with d = kk*256 + i*128 + p."""
    d, cols = mat_t.shape
    return np.ascontiguousarray(
        mat_t.reshape(KT8, 2, 128, cols).transpose(2, 0, 1, 3)
    )


def kernel(x, op_ids, n_op):
    global LAST_RESULT
    from concourse.bass_utils import run_bass_kernel_spmd

    x = np.asarray(x, dtype=np.float32).reshape(-1, D)
    op_ids = np.asarray(op_ids).reshape(-1).astype(np.int64)
    n_op_i = int(np.asarray(n_op))

    # ---- host prep: normalize, quantize, class sums, diagonal ----
    norms = np.sqrt((x.astype(np.float64) ** 2).sum(axis=1))
    norms = np.maximum(norms, EPS).astype(np.float32)
    z = x / norms[:, None]

    z8 = z.astype(FP8)
    z8f = z8.astype(np.float32)

    onehot = np.zeros((N, NOP), np.float32)
    onehot[np.arange(N), op_ids] = 1.0
    W8 = (onehot.T @ z8f).astype(FP8)               # [NOP, D] fp8

    z8_packed = _pack_dr(np.ascontiguousarray(z8.T))          # [128,3,2,N]
    w8_packed = _pack_dr(np.ascontiguousarray(W8.T.astype(FP8)))
    ssq = (z8f.astype(np.float64) ** 2).sum(axis=1)  # = sim[i, i]

    in_maps = []
    for c in range(CORES):
        # rows 1024c..1024c+1023 as [128, NROW, KT8, 2, 128] lhsT blocks
        a8_c = np.ascontiguousarray(
            z8_packed[:, :, :, 1024 * c:1024 * (c + 1)]
            .reshape(128, KT8, 2, NROW, 128)
            .transpose(0, 3, 1, 2, 4)
        )
        # rotated z8 columns [0, ZCOLS) as NGZ contiguous groups
        idx = (np.arange(ZCOLS) + 1024 * c) % N
        z8_c = np.ascontiguousarray(
            z8_packed[:, :, :, idx]
            .reshape(128, KT8, 2, NGZ, GW)
            .transpose(0, 3, 1, 2, 4)
        )
        in_maps.append({"z8": z8_c, "a8": a8_c, "w8": w8_packed})

    nc = _get_nc()
    res = run_bass_kernel_spmd(nc, in_maps, core_ids=list(range(CORES)))
    LAST_RESULT = res

    # ---- host post: assemble es = rowsums + colsums, finish loss ----
    es = np.zeros(N, np.float64)
    tm_full = np.zeros((N, NOP), np.float64)
    for c in range(CORES):
        r = res.results[c]
        pacc_c = r["pacc"].astype(np.float64)      # [128, NROW, 4]
        tm_c = r["tm"].astype(np.float64)          # [128, NROW, NOP]
        cs = np.zeros(N, np.float64)
        cs[:ACCW] = r["acc_out"].astype(np.float64).sum(axis=0)
        es += np.roll(cs, 1024 * c)                # unrotate
        for k in range(NROW):
            t = 8 * c + k
            rows = slice(128 * t, 128 * t + 128)
            es[rows] += pacc_c[:, k, :].sum(axis=1)
            tm_full[rows] = tm_c[:, k, :]

    lse = np.log(es - np.exp(TEMP_INV * ssq))
    pos_sum = TEMP_INV * (tm_full[np.arange(N), op_ids] - ssq)
    counts = np.bincount(op_ids, minlength=n_op_i).astype(np.float64)
    pos_cnt = counts[op_ids] - 1.0

    loss_i = np.where(pos_cnt > 0, -pos_sum / np.maximum(pos_cnt, 1.0) + lse, 0.0)
    cls_sum = np.bincount(op_ids, weights=loss_i, minlength=n_op_i)
    cls_loss = np.where(counts > 0, cls_sum / np.maximum(counts, 1.0), 0.0)
    return np.float32(cls_loss.mean())
